# revision 28
# baseline (speedup 1.0000x reference)
"""Causal self-attention (B=2, T=2048, C=2048, H=16, D=128) on 8 trn2 cores.

Sharding: tensor-parallel over heads x data-parallel over batch.
Core c handles batch c//4, heads [4*(c%4) .. 4*(c%4)+4). Each core computes
qkv projection for its 4 heads, RoPE, causal attention, and a partial
output projection (its heads' rows of W_proj); the host sums the 4 partials
per batch.

All matmul operands are bf16 (fp32 PSUM accumulation): same PE streaming
rate as fp32r at N=512 but 4x faster weight loads (FWL) and half the DMA
traffic. q/k/v stay RESIDENT in SBUF between phases (48KB/partition) - no
DRAM spill round-trip. Weights are pre-transposed on the host so every DMA
is dense.

Phase 2 per head processes score-tile pairs [128,1024] (2 PSUM banks, one
ACT exp per pair), software-pipelined two pairs deep so the in-order PE
queue always has S matmuls to run while ACT exponentiates. Diagonal tiles
are width-restricted to the causally valid q-range (saves ~15% of phase-2
matmul columns and exp elements); the causal boundary inside a tile is a
single [128,128] additive triangle mask.
"""

import math
import os

import numpy as np

B, T, C = 2, 2048, 2048
H, D = 16, 128
HPC = 4  # heads per core
NCORES = 8

_CACHE = {}


def _build_program():
    import concourse.tile as tile
    from concourse import bacc, mybir

    f32 = mybir.dt.float32
    bf16 = mybir.dt.bfloat16
    Exp = mybir.ActivationFunctionType.Exp
    SCALE = 1.0 / math.sqrt(float(D))

    nc = bacc.Bacc(
        "TRN2", target_bir_lowering=False, debug=False, num_devices=NCORES
    )

    KT = C // 128  # 16 contraction tiles
    NTB = T // 512  # 4 t-blocks

    xT = nc.dram_tensor("xT", [C, T], bf16, kind="ExternalInput").ap()
    # [128, m, k*128]: per-m dense columns of the q|k weight blocks
    wqkT = nc.dram_tensor(
        "wqkT", [128, 8, KT * 128], bf16, kind="ExternalInput"
    ).ap()
    # [128, k*512]: per-k dense chunks of the v weight block
    wvT = nc.dram_tensor("wvT", [128, KT * 512], bf16, kind="ExternalInput").ap()
    wp = nc.dram_tensor("wp", [HPC * D, C], bf16, kind="ExternalInput").ap()
    onesr = nc.dram_tensor("onesr", [128, 128], bf16, kind="ExternalInput").ap()
    cosT = nc.dram_tensor("cosT", [128, T], bf16, kind="ExternalInput").ap()
    sinTs = nc.dram_tensor("sinTs", [128, T], bf16, kind="ExternalInput").ap()
    masktri = nc.dram_tensor("masktri", [128, 128], f32, kind="ExternalInput").ap()
    out = nc.dram_tensor("out", [T, C], bf16, kind="ExternalOutput").ap()

    with tile.TileContext(nc) as tc:
        with (
            tc.tile_pool(name="consts", bufs=1) as consts,
            tc.tile_pool(name="resid", bufs=1) as resid,
        ):
            # consts tiles are allocated here but their DMAs are emitted
            # inside phase 1, after the critical-path weight/x loads, so
            # they don't delay the first matmul chain
            cos_sb = consts.tile([128, T], bf16, tag="cos")
            sin_sb = consts.tile([128, T], bf16, tag="sin")
            ones_sb = consts.tile([128, 128], bf16, tag="ones")
            tri_sb = consts.tile([128, 128], f32, tag="tri")

            # q/k/v resident between phases: [128, m, T] for Q^T/K^T rows
            # (m 0-3 = q heads, 4-7 = k heads), [128, tchunk, 4*D] for V.
            qkres = resid.tile([128, 8, T], bf16, tag="qkres")
            vres = resid.tile([128, KT, HPC * D], bf16, tag="vres")
            o2 = [
                resid.tile([128, T], bf16, tag=f"o2_{h}", name=f"o2_{h}")
                for h in range(HPC)
            ]

            # ---------------- Phase 1: QKV projection ----------------
            with (
                tc.tile_pool(name="p1x", bufs=2) as p1x,
                tc.tile_pool(name="p1w", bufs=1) as p1w,
                tc.tile_pool(name="p1wv", bufs=1) as p1wv,
                tc.tile_pool(name="p1e", bufs=2) as p1e,
                tc.tile_pool(name="p1ps", bufs=2, space="PSUM") as p1ps,
            ):
                wqkg = p1w.tile([128, 8, KT * 128], bf16, tag="wqkg")
                wv_sb = p1wv.tile([128, KT * 512], bf16, tag="wv")
                xtb0 = p1x.tile([128, KT, 512], bf16, tag="xtb")
                MORD = (0, 4, 1, 5, 2, 6, 3, 7)

                def load_wm(m, eng=None):
                    (eng or nc.sync).dma_start(out=wqkg[:, m], in_=wqkT[:, m])

                # consts on the scalar queue (DVE needs cos/sin by the first
                # RoPE evacuation); weight/x critical path on the sync queue.
                nc.scalar.dma_start(out=cos_sb, in_=cosT)
                nc.scalar.dma_start(out=sin_sb, in_=sinTs)
                nc.gpsimd.dma_start(out=ones_sb, in_=onesr)
                nc.gpsimd.dma_start(out=tri_sb, in_=masktri)
                # first quarter of wm0 (k0-3) lands fast so the first
                # matmul starts earlier; rest of wm0 streams behind the
                # first x chunks it will be consumed with
                wm0_dram = wqkT[:, 0]
                wm0_sb = wqkg[:, 0]
                nc.sync.dma_start(out=wm0_sb[:, 0:512], in_=wm0_dram[:, 0:512])
                nc.sync.dma_start(out=xtb0[:, 0], in_=xT[0:128, 0:512])
                nc.sync.dma_start(out=xtb0[:, 1], in_=xT[128:256, 0:512])
                nc.sync.dma_start(
                    out=wm0_sb[:, 512:2048], in_=wm0_dram[:, 512:2048]
                )
                load_wm(MORD[1])
                load_wm(MORD[2])
                for k in range(2, KT):
                    nc.sync.dma_start(
                        out=xtb0[:, k], in_=xT[k * 128 : (k + 1) * 128, 0:512]
                    )
                    if k % 2 == 0 and k // 2 + 2 < 8:
                        load_wm(MORD[k // 2 + 2])
                for k in range(KT):
                    ks = slice(k * 512, (k + 1) * 512)
                    nc.gpsimd.dma_start(out=wv_sb[:, ks], in_=wvT[:, ks])
                for tb in range(NTB):
                    tsl = slice(tb * 512, (tb + 1) * 512)
                    if tb == 0:
                        xtb = xtb0
                    else:
                        xtb = p1x.tile([128, KT, 512], bf16, tag="xtb",
                                       name="xtb")
                        for k in range(KT):
                            nc.sync.dma_start(
                                out=xtb[:, k],
                                in_=xT[k * 128 : (k + 1) * 128, tsl],
                            )
                    for m in MORD:
                        ps = p1ps.tile([128, 512], f32, tag="qk")
                        for k in range(KT):
                            nc.tensor.matmul(
                                ps,
                                lhsT=wqkg[:, m, k * 128 : (k + 1) * 128],
                                rhs=xtb[:, k, :],
                                start=(k == 0),
                                stop=(k == KT - 1),
                            )
                        # RoPE fused with PSUM evacuation, direct to qkres.
                        tmp = p1e.tile([128, 512], f32, tag="rtmp")
                        nc.vector.tensor_mul(
                            tmp[0:64], ps[64:128], sin_sb[0:64, tsl]
                        )
                        nc.vector.tensor_mul(
                            tmp[64:128], ps[0:64], sin_sb[64:128, tsl]
                        )
                        tmp2 = p1e.tile([128, 512], f32, tag="rtmp2")
                        nc.vector.tensor_mul(tmp2, ps, cos_sb[:, tsl])
                        nc.vector.tensor_add(qkres[:, m, tsl], tmp2, tmp)
                    for tsub in range(4):
                        csl = slice(tsub * 128, (tsub + 1) * 128)
                        psv = p1ps.tile([128, 512], f32, tag="v", bufs=2)
                        for k in range(KT):
                            nc.tensor.matmul(
                                psv,
                                lhsT=xtb[:, k, csl],
                                rhs=wv_sb[:, k * 512 : (k + 1) * 512],
                                start=(k == 0),
                                stop=(k == KT - 1),
                            )
                        nc.scalar.copy(vres[:, tb * 4 + tsub, :], psv)

            # ---------------- Phase 2: attention ----------------
            with tc.tile_pool(name="p3w", bufs=1) as p3w:
                wps = [
                    p3w.tile([128, T], bf16, tag=f"wp{i}", name=f"wp{i}")
                    for i in range(HPC)
                ]
                for i in range(HPC):
                    nc.sync.dma_start(
                        out=wps[i], in_=wp[i * 128 : (i + 1) * 128, :]
                    )
                _phase2(tc, nc, f32, bf16, Exp, SCALE, qkres, vres,
                        tri_sb, ones_sb, o2)
                _phase3(tc, nc, f32, bf16, o2, wps, out)
    nc.compile()
    return nc


def _phase2(tc, nc, f32, bf16, Exp, SCALE, qkres, vres, tri_sb, ones_sb, o2):
    """Score pairs software-pipelined 2 deep: S matmuls (and exp) for pair
    j+2 are emitted before pv/dn matmuls of pair j, so the in-order PE
    stream never waits on ACT."""
    with (
        tc.tile_pool(name="p2st", bufs=3, space="PSUM") as stp,
        tc.tile_pool(name="p2pt", bufs=3) as ptp,
        tc.tile_pool(name="p2s", bufs=2) as p2s,
        tc.tile_pool(name="p2pv", bufs=1, space="PSUM") as pvp,
        tc.tile_pool(name="p2dn", bufs=1, space="PSUM") as dnp,
    ):
        # flat pair list across heads and q-blocks
        pairs = []
        for h in range(HPC):
            for qb in range(4):
                nk = 4 * (qb + 1)
                for jp in range(nk // 2):
                    pairs.append((h, qb, nk, 2 * jp, 2 * jp + 1))

        def width(qb, kb):
            j = kb - 4 * qb
            return 512 if j < 0 else 512 - 128 * j

        queue = []
        state = {}  # (h, qb) -> (pv, dn)
        for idx in range(len(pairs) + 2):
            if idx < len(pairs):
                h, qb, nk, kb0, kb1 = pairs[idx]
                w0, w1 = width(qb, kb0), width(qb, kb1)
                st = stp.tile([128, 1024], f32, tag="st")
                pt = ptp.tile([128, 1024], bf16, tag="pt")
                for kb, w, o in ((kb0, w0, 0), (kb1, w1, w0)):
                    off = 512 - w
                    nc.tensor.matmul(
                        st[:, o : o + w],
                        lhsT=qkres[:, 4 + h, kb * 128 : (kb + 1) * 128],
                        rhs=qkres[:, h, qb * 512 + off : (qb + 1) * 512],
                        start=True,
                        stop=True,
                    )
                    if kb >= 4 * qb:  # diagonal: causal triangle mask
                        nc.vector.tensor_add(
                            st[:, o : o + 128], st[:, o : o + 128], tri_sb
                        )
                nc.scalar.activation(
                    pt[:, 0 : w0 + w1], st[:, 0 : w0 + w1], Exp, scale=SCALE
                )
                queue.append((h, qb, nk, pt, ((kb0, w0, 0), (kb1, w1, w0))))
            if idx >= 2:
                h, qb, nk, pt, halves = queue[idx - 2]
                if halves[0][0] == 0:  # first pair of this q-block
                    state[(h, qb)] = (
                        pvp.tile([128, 512], f32, tag="pv",
                                 name=f"pv{h}_{qb}"),
                        dnp.tile([128, 512], f32, tag="dn",
                                 name=f"dn{h}_{qb}"),
                    )
                pv, dn = state[(h, qb)]
                for kb, w, o in halves:
                    off = 512 - w
                    nc.tensor.matmul(
                        dn[:, off:512],
                        lhsT=ones_sb,
                        rhs=pt[:, o : o + w],
                        start=(kb == 0),
                        stop=(kb == nk - 1),
                    )
                    nc.tensor.matmul(
                        pv[:, off:512],
                        lhsT=vres[:, kb, h * 128 : (h + 1) * 128],
                        rhs=pt[:, o : o + w],
                        start=(kb == 0),
                        stop=(kb == nk - 1),
                    )
                if halves[1][0] == nk - 1:  # last pair: normalize + evac
                    qsl = slice(qb * 512, (qb + 1) * 512)
                    rb2 = p2s.tile([128, 512], f32, tag="rb2")
                    nc.vector.reciprocal_approx_fast(out=rb2, in_=dn)
                    nc.vector.tensor_mul(o2[h][:, qsl], pv, rb2)


def _phase3(tc, nc, f32, bf16, o2, wps, out):
    with (
        tc.tile_pool(name="p3e", bufs=4) as p3e,
        tc.tile_pool(name="p3ps", bufs=8, space="PSUM") as p3ps,
    ):
        NT = T // 128
        for t in range(NT):
            tsl = slice(t * 128, (t + 1) * 128)
            pos = [
                p3ps.tile([128, 512], f32, tag="po", name=f"po{t}_{cb}")
                for cb in range(4)
            ]
            if t < NT - 1:
                # hd outer / cb inner: 4 matmuls share one LDWEIGHTS.
                for hd in range(HPC):
                    for cb in range(4):
                        nc.tensor.matmul(
                            pos[cb],
                            lhsT=o2[hd][:, tsl],
                            rhs=wps[hd][:, cb * 512 : (cb + 1) * 512],
                            start=(hd == 0),
                            stop=(hd == HPC - 1),
                        )
                for cb in range(4):
                    ob = p3e.tile([128, 512], bf16, tag="ob")
                    # alternate evacuation engine: DVE / ACT
                    if cb % 2 == 0:
                        nc.vector.tensor_copy(ob, pos[cb])
                    else:
                        nc.scalar.copy(ob, pos[cb])
                    nc.sync.dma_start(
                        out=out[tsl, cb * 512 : (cb + 1) * 512], in_=ob
                    )
            else:
                # last tile: cb outer so each pos finishes early and its
                # evacuation + DMA overlap the remaining matmuls (short tail)
                for cb in range(4):
                    for hd in range(HPC):
                        nc.tensor.matmul(
                            pos[cb],
                            lhsT=o2[hd][:, tsl],
                            rhs=wps[hd][:, cb * 512 : (cb + 1) * 512],
                            start=(hd == 0),
                            stop=(hd == HPC - 1),
                        )
                    ob = p3e.tile([128, 512], bf16, tag="ob")
                    if cb % 2 == 0:
                        nc.vector.tensor_copy(ob, pos[cb])
                    else:
                        nc.scalar.copy(ob, pos[cb])
                    nc.sync.dma_start(
                        out=out[tsl, cb * 512 : (cb + 1) * 512], in_=ob
                    )


def _get_program():
    if "nc" not in _CACHE:
        _CACHE["nc"] = _build_program()
    return _CACHE["nc"]


def make_in_maps(x, cos, sin, W_qkv, W_proj):
    """Host-side sharding: per-core input dicts (numpy)."""
    import ml_dtypes

    bf16 = ml_dtypes.bfloat16
    KT = C // 128
    x = np.asarray(x, dtype=np.float32)
    cos = np.asarray(cos, dtype=np.float32)
    sin = np.asarray(sin, dtype=np.float32)
    W_qkv = np.asarray(W_qkv, dtype=np.float32)
    W_proj = np.asarray(W_proj, dtype=np.float32)

    cosT = np.ascontiguousarray(np.tile(cos.T, (2, 1)).astype(bf16))  # [128, T]
    sinT = np.ascontiguousarray(
        np.concatenate([-sin.T, sin.T], axis=0).astype(bf16)
    )
    k_idx = np.arange(128)[:, None]
    q_idx = np.arange(128)[None, :]
    masktri = np.where(q_idx >= k_idx, 0.0, -1.0e30).astype(np.float32)
    onesr = np.ones((128, 128), dtype=bf16)

    in_maps = []
    for core in range(NCORES):
        b, hg = core // 4, core % 4
        csl = slice(hg * 512, (hg + 1) * 512)
        # q|k columns for this head group: [C, 1024]
        wqk_np = np.concatenate(
            [W_qkv[:, csl], W_qkv[:, C + hg * 512 : C + (hg + 1) * 512]],
            axis=1,
        )
        # -> [128(p), 8(m), KT*128] with rows split as (k,p)
        wqkT_np = np.ascontiguousarray(
            wqk_np.reshape(KT, 128, 8, 128)
            .transpose(1, 2, 0, 3)
            .reshape(128, 8, KT * 128)
            .astype(bf16)
        )
        wv_np = W_qkv[:, 2 * C + hg * 512 : 2 * C + (hg + 1) * 512]
        wvT_np = np.ascontiguousarray(
            wv_np.reshape(KT, 128, 512)
            .transpose(1, 0, 2)
            .reshape(128, KT * 512)
            .astype(bf16)
        )
        wp_np = np.ascontiguousarray(
            W_proj[hg * 512 : (hg + 1) * 512, :].astype(bf16)
        )
        xT_np = np.ascontiguousarray(x[b].T.astype(bf16))
        in_maps.append(
            {
                "xT": xT_np,
                "wqkT": wqkT_np,
                "wvT": wvT_np,
                "wp": wp_np,
                "onesr": onesr,
                "cosT": cosT,
                "sinTs": sinT,
                "masktri": masktri,
            }
        )
    return in_maps


def kernel(x, cos, sin, W_qkv, W_proj):
    from concourse.bass_utils import run_bass_kernel_spmd

    nc = _get_program()
    in_maps = make_in_maps(x, cos, sin, W_qkv, W_proj)
    trace = bool(int(os.environ.get("KERNEL_TRACE", "0")))
    res = run_bass_kernel_spmd(
        nc, in_maps, core_ids=list(range(NCORES)), trace=trace
    )
    if trace:
        _CACHE["last_results"] = res
        if res.exec_time_ns is not None:
            print(f"HW exec time: {res.exec_time_ns} ns")

    out = np.zeros((B, T, C), dtype=np.float32)
    for core in range(NCORES):
        out[core // 4] += res.results[core]["out"].astype(np.float32)
    return out


# revision 29
# speedup vs baseline: 1.0154x; 1.0154x over previous
"""Causal self-attention (B=2, T=2048, C=2048, H=16, D=128) on 8 trn2 cores.

Sharding: tensor-parallel over heads x data-parallel over batch.
Core c handles batch c//4, heads [4*(c%4) .. 4*(c%4)+4). Each core computes
qkv projection for its 4 heads, RoPE, causal attention, and a partial
output projection (its heads' rows of W_proj); the host sums the 4 partials
per batch.

All matmul operands are bf16 (fp32 PSUM accumulation): same PE streaming
rate as fp32r at N=512 but 4x faster weight loads (FWL) and half the DMA
traffic. q/k/v stay RESIDENT in SBUF between phases (48KB/partition) - no
DRAM spill round-trip. Weights are pre-transposed on the host so every DMA
is dense.

Phase 2 per head processes score-tile pairs [128,1024] (2 PSUM banks, one
ACT exp per pair), software-pipelined two pairs deep so the in-order PE
queue always has S matmuls to run while ACT exponentiates. Diagonal tiles
are width-restricted to the causally valid q-range (saves ~15% of phase-2
matmul columns and exp elements); the causal boundary inside a tile is a
single [128,128] additive triangle mask.
"""

import math
import os

import numpy as np

B, T, C = 2, 2048, 2048
H, D = 16, 128
HPC = 4  # heads per core
NCORES = 8

_CACHE = {}


def _build_program():
    import concourse.tile as tile
    from concourse import bacc, mybir

    f32 = mybir.dt.float32
    bf16 = mybir.dt.bfloat16
    Exp = mybir.ActivationFunctionType.Exp
    SCALE = 1.0 / math.sqrt(float(D))

    nc = bacc.Bacc(
        "TRN2", target_bir_lowering=False, debug=False, num_devices=NCORES
    )

    KT = C // 128  # 16 contraction tiles
    NTB = T // 512  # 4 t-blocks

    xT = nc.dram_tensor("xT", [C, T], bf16, kind="ExternalInput").ap()
    # [128, m, k*128]: per-m dense columns of the q|k weight blocks
    wqkT = nc.dram_tensor(
        "wqkT", [128, 8, KT * 128], bf16, kind="ExternalInput"
    ).ap()
    # [128, k*512]: per-k dense chunks of the v weight block
    wvT = nc.dram_tensor("wvT", [128, KT * 512], bf16, kind="ExternalInput").ap()
    wp = nc.dram_tensor("wp", [HPC * D, C], bf16, kind="ExternalInput").ap()
    onesr = nc.dram_tensor("onesr", [128, 128], bf16, kind="ExternalInput").ap()
    cosT = nc.dram_tensor("cosT", [128, T], bf16, kind="ExternalInput").ap()
    sinTs = nc.dram_tensor("sinTs", [128, T], bf16, kind="ExternalInput").ap()
    masktri = nc.dram_tensor("masktri", [128, 128], f32, kind="ExternalInput").ap()
    out = nc.dram_tensor("out", [T, C], bf16, kind="ExternalOutput").ap()

    with tile.TileContext(nc) as tc:
        with (
            tc.tile_pool(name="consts", bufs=1) as consts,
            tc.tile_pool(name="resid", bufs=1) as resid,
        ):
            # consts tiles are allocated here but their DMAs are emitted
            # inside phase 1, after the critical-path weight/x loads, so
            # they don't delay the first matmul chain
            cos_sb = consts.tile([128, T], bf16, tag="cos")
            sin_sb = consts.tile([128, T], bf16, tag="sin")
            ones_sb = consts.tile([128, 128], bf16, tag="ones")
            tri_sb = consts.tile([128, 128], f32, tag="tri")

            # q/k/v resident between phases: [128, m, T] for Q^T/K^T rows
            # (m 0-3 = q heads, 4-7 = k heads), [128, tchunk, 4*D] for V.
            qkres = resid.tile([128, 8, T], bf16, tag="qkres")
            vres = resid.tile([128, KT, HPC * D], bf16, tag="vres")
            o2 = [
                resid.tile([128, T], bf16, tag=f"o2_{h}", name=f"o2_{h}")
                for h in range(HPC)
            ]

            # ---------------- Phase 1: QKV projection ----------------
            with (
                tc.tile_pool(name="p1x", bufs=2) as p1x,
                tc.tile_pool(name="p1w", bufs=1) as p1w,
                tc.tile_pool(name="p1wv", bufs=1) as p1wv,
                tc.tile_pool(name="p1e", bufs=2) as p1e,
                tc.tile_pool(name="p1ps", bufs=2, space="PSUM") as p1ps,
            ):
                wqkg = p1w.tile([128, 8, KT * 128], bf16, tag="wqkg")
                wv_sb = p1wv.tile([128, KT * 512], bf16, tag="wv")
                xtb0 = p1x.tile([128, KT, 512], bf16, tag="xtb")
                MORD = (0, 4, 1, 5, 2, 6, 3, 7)

                def load_wm(m, eng=None):
                    (eng or nc.sync).dma_start(out=wqkg[:, m], in_=wqkT[:, m])

                # consts on the scalar queue (DVE needs cos/sin by the first
                # RoPE evacuation); weight/x critical path on the sync queue.
                nc.scalar.dma_start(out=cos_sb, in_=cosT)
                nc.scalar.dma_start(out=sin_sb, in_=sinTs)
                nc.scalar.dma_start(out=ones_sb, in_=onesr)
                nc.scalar.dma_start(out=tri_sb, in_=masktri)
                load_wm(MORD[0])
                load_wm(MORD[1])
                for k in range(KT):
                    nc.sync.dma_start(
                        out=xtb0[:, k], in_=xT[k * 128 : (k + 1) * 128, 0:512]
                    )
                    if k % 2 == 0 and k // 2 + 2 < 8:
                        load_wm(MORD[k // 2 + 2])
                for k in range(KT):
                    ks = slice(k * 512, (k + 1) * 512)
                    nc.sync.dma_start(out=wv_sb[:, ks], in_=wvT[:, ks])
                for tb in range(NTB):
                    tsl = slice(tb * 512, (tb + 1) * 512)
                    if tb == 0:
                        xtb = xtb0
                    else:
                        xtb = p1x.tile([128, KT, 512], bf16, tag="xtb",
                                       name="xtb")
                        for k in range(KT):
                            nc.sync.dma_start(
                                out=xtb[:, k],
                                in_=xT[k * 128 : (k + 1) * 128, tsl],
                            )
                    for m in MORD:
                        ps = p1ps.tile([128, 512], f32, tag="qk")
                        for k in range(KT):
                            nc.tensor.matmul(
                                ps,
                                lhsT=wqkg[:, m, k * 128 : (k + 1) * 128],
                                rhs=xtb[:, k, :],
                                start=(k == 0),
                                stop=(k == KT - 1),
                            )
                        # RoPE fused with PSUM evacuation, direct to qkres.
                        tmp = p1e.tile([128, 512], f32, tag="rtmp")
                        nc.vector.tensor_mul(
                            tmp[0:64], ps[64:128], sin_sb[0:64, tsl]
                        )
                        nc.vector.tensor_mul(
                            tmp[64:128], ps[0:64], sin_sb[64:128, tsl]
                        )
                        tmp2 = p1e.tile([128, 512], f32, tag="rtmp2")
                        nc.vector.tensor_mul(tmp2, ps, cos_sb[:, tsl])
                        nc.vector.tensor_add(qkres[:, m, tsl], tmp2, tmp)
                    for tsub in range(4):
                        csl = slice(tsub * 128, (tsub + 1) * 128)
                        psv = p1ps.tile([128, 512], f32, tag="v", bufs=2)
                        for k in range(KT):
                            nc.tensor.matmul(
                                psv,
                                lhsT=xtb[:, k, csl],
                                rhs=wv_sb[:, k * 512 : (k + 1) * 512],
                                start=(k == 0),
                                stop=(k == KT - 1),
                            )
                        nc.scalar.copy(vres[:, tb * 4 + tsub, :], psv)

            # ---------------- Phase 2: attention ----------------
            with tc.tile_pool(name="p3w", bufs=1) as p3w:
                wps = [
                    p3w.tile([128, T], bf16, tag=f"wp{i}", name=f"wp{i}")
                    for i in range(HPC)
                ]
                for i in range(HPC):
                    nc.sync.dma_start(
                        out=wps[i], in_=wp[i * 128 : (i + 1) * 128, :]
                    )
                _phase2(tc, nc, f32, bf16, Exp, SCALE, qkres, vres,
                        tri_sb, ones_sb, o2)
                _phase3(tc, nc, f32, bf16, o2, wps, out)
    nc.compile()
    return nc


def _phase2(tc, nc, f32, bf16, Exp, SCALE, qkres, vres, tri_sb, ones_sb, o2):
    """Score pairs software-pipelined 2 deep: S matmuls (and exp) for pair
    j+2 are emitted before pv/dn matmuls of pair j, so the in-order PE
    stream never waits on ACT."""
    with (
        tc.tile_pool(name="p2st", bufs=3, space="PSUM") as stp,
        tc.tile_pool(name="p2pt", bufs=3) as ptp,
        tc.tile_pool(name="p2s", bufs=2) as p2s,
        tc.tile_pool(name="p2pv", bufs=1, space="PSUM") as pvp,
        tc.tile_pool(name="p2dn", bufs=1, space="PSUM") as dnp,
    ):
        # flat pair list across heads and q-blocks
        pairs = []
        for h in range(HPC):
            for qb in range(4):
                nk = 4 * (qb + 1)
                for jp in range(nk // 2):
                    pairs.append((h, qb, nk, 2 * jp, 2 * jp + 1))

        def width(qb, kb):
            j = kb - 4 * qb
            return 512 if j < 0 else 512 - 128 * j

        queue = []
        state = {}  # (h, qb) -> (pv, dn)
        for idx in range(len(pairs) + 2):
            if idx < len(pairs):
                h, qb, nk, kb0, kb1 = pairs[idx]
                w0, w1 = width(qb, kb0), width(qb, kb1)
                st = stp.tile([128, 1024], f32, tag="st")
                pt = ptp.tile([128, 1024], bf16, tag="pt")
                for kb, w, o in ((kb0, w0, 0), (kb1, w1, w0)):
                    off = 512 - w
                    nc.tensor.matmul(
                        st[:, o : o + w],
                        lhsT=qkres[:, 4 + h, kb * 128 : (kb + 1) * 128],
                        rhs=qkres[:, h, qb * 512 + off : (qb + 1) * 512],
                        start=True,
                        stop=True,
                    )
                    if kb >= 4 * qb:  # diagonal: causal triangle mask
                        nc.vector.tensor_add(
                            st[:, o : o + 128], st[:, o : o + 128], tri_sb
                        )
                nc.scalar.activation(
                    pt[:, 0 : w0 + w1], st[:, 0 : w0 + w1], Exp, scale=SCALE
                )
                queue.append((h, qb, nk, pt, ((kb0, w0, 0), (kb1, w1, w0))))
            if idx >= 2:
                h, qb, nk, pt, halves = queue[idx - 2]
                if halves[0][0] == 0:  # first pair of this q-block
                    state[(h, qb)] = (
                        pvp.tile([128, 512], f32, tag="pv",
                                 name=f"pv{h}_{qb}"),
                        dnp.tile([128, 512], f32, tag="dn",
                                 name=f"dn{h}_{qb}"),
                    )
                pv, dn = state[(h, qb)]
                for kb, w, o in halves:
                    off = 512 - w
                    nc.tensor.matmul(
                        dn[:, off:512],
                        lhsT=ones_sb,
                        rhs=pt[:, o : o + w],
                        start=(kb == 0),
                        stop=(kb == nk - 1),
                    )
                    nc.tensor.matmul(
                        pv[:, off:512],
                        lhsT=vres[:, kb, h * 128 : (h + 1) * 128],
                        rhs=pt[:, o : o + w],
                        start=(kb == 0),
                        stop=(kb == nk - 1),
                    )
                if halves[1][0] == nk - 1:  # last pair: normalize + evac
                    qsl = slice(qb * 512, (qb + 1) * 512)
                    rb2 = p2s.tile([128, 512], f32, tag="rb2")
                    nc.vector.reciprocal_approx_fast(out=rb2, in_=dn)
                    nc.vector.tensor_mul(o2[h][:, qsl], pv, rb2)


def _phase3(tc, nc, f32, bf16, o2, wps, out):
    with (
        tc.tile_pool(name="p3e", bufs=4) as p3e,
        tc.tile_pool(name="p3ps", bufs=8, space="PSUM") as p3ps,
    ):
        NT = T // 128
        for t in range(NT):
            tsl = slice(t * 128, (t + 1) * 128)
            pos = [
                p3ps.tile([128, 512], f32, tag="po", name=f"po{t}_{cb}")
                for cb in range(4)
            ]
            if t < NT - 1:
                # hd outer / cb inner: 4 matmuls share one LDWEIGHTS.
                for hd in range(HPC):
                    for cb in range(4):
                        nc.tensor.matmul(
                            pos[cb],
                            lhsT=o2[hd][:, tsl],
                            rhs=wps[hd][:, cb * 512 : (cb + 1) * 512],
                            start=(hd == 0),
                            stop=(hd == HPC - 1),
                        )
                for cb in range(4):
                    ob = p3e.tile([128, 512], bf16, tag="ob")
                    # alternate evacuation engine: DVE / ACT
                    if cb % 2 == 0:
                        nc.vector.tensor_copy(ob, pos[cb])
                    else:
                        nc.scalar.copy(ob, pos[cb])
                    nc.sync.dma_start(
                        out=out[tsl, cb * 512 : (cb + 1) * 512], in_=ob
                    )
            else:
                # last tile: cb outer so each pos finishes early and its
                # evacuation + DMA overlap the remaining matmuls (short tail)
                for cb in range(4):
                    for hd in range(HPC):
                        nc.tensor.matmul(
                            pos[cb],
                            lhsT=o2[hd][:, tsl],
                            rhs=wps[hd][:, cb * 512 : (cb + 1) * 512],
                            start=(hd == 0),
                            stop=(hd == HPC - 1),
                        )
                    ob = p3e.tile([128, 512], bf16, tag="ob")
                    if cb % 2 == 0:
                        nc.vector.tensor_copy(ob, pos[cb])
                    else:
                        nc.scalar.copy(ob, pos[cb])
                    nc.sync.dma_start(
                        out=out[tsl, cb * 512 : (cb + 1) * 512], in_=ob
                    )


def _get_program():
    if "nc" not in _CACHE:
        _CACHE["nc"] = _build_program()
    return _CACHE["nc"]


def make_in_maps(x, cos, sin, W_qkv, W_proj):
    """Host-side sharding: per-core input dicts (numpy)."""
    import ml_dtypes

    bf16 = ml_dtypes.bfloat16
    KT = C // 128
    x = np.asarray(x, dtype=np.float32)
    cos = np.asarray(cos, dtype=np.float32)
    sin = np.asarray(sin, dtype=np.float32)
    W_qkv = np.asarray(W_qkv, dtype=np.float32)
    W_proj = np.asarray(W_proj, dtype=np.float32)

    cosT = np.ascontiguousarray(np.tile(cos.T, (2, 1)).astype(bf16))  # [128, T]
    sinT = np.ascontiguousarray(
        np.concatenate([-sin.T, sin.T], axis=0).astype(bf16)
    )
    k_idx = np.arange(128)[:, None]
    q_idx = np.arange(128)[None, :]
    masktri = np.where(q_idx >= k_idx, 0.0, -1.0e30).astype(np.float32)
    onesr = np.ones((128, 128), dtype=bf16)

    in_maps = []
    for core in range(NCORES):
        b, hg = core // 4, core % 4
        csl = slice(hg * 512, (hg + 1) * 512)
        # q|k columns for this head group: [C, 1024]
        wqk_np = np.concatenate(
            [W_qkv[:, csl], W_qkv[:, C + hg * 512 : C + (hg + 1) * 512]],
            axis=1,
        )
        # -> [128(p), 8(m), KT*128] with rows split as (k,p)
        wqkT_np = np.ascontiguousarray(
            wqk_np.reshape(KT, 128, 8, 128)
            .transpose(1, 2, 0, 3)
            .reshape(128, 8, KT * 128)
            .astype(bf16)
        )
        wv_np = W_qkv[:, 2 * C + hg * 512 : 2 * C + (hg + 1) * 512]
        wvT_np = np.ascontiguousarray(
            wv_np.reshape(KT, 128, 512)
            .transpose(1, 0, 2)
            .reshape(128, KT * 512)
            .astype(bf16)
        )
        wp_np = np.ascontiguousarray(
            W_proj[hg * 512 : (hg + 1) * 512, :].astype(bf16)
        )
        xT_np = np.ascontiguousarray(x[b].T.astype(bf16))
        in_maps.append(
            {
                "xT": xT_np,
                "wqkT": wqkT_np,
                "wvT": wvT_np,
                "wp": wp_np,
                "onesr": onesr,
                "cosT": cosT,
                "sinTs": sinT,
                "masktri": masktri,
            }
        )
    return in_maps


def kernel(x, cos, sin, W_qkv, W_proj):
    from concourse.bass_utils import run_bass_kernel_spmd

    nc = _get_program()
    in_maps = make_in_maps(x, cos, sin, W_qkv, W_proj)
    trace = bool(int(os.environ.get("KERNEL_TRACE", "0")))
    res = run_bass_kernel_spmd(
        nc, in_maps, core_ids=list(range(NCORES)), trace=trace
    )
    if trace:
        _CACHE["last_results"] = res
        if res.exec_time_ns is not None:
            print(f"HW exec time: {res.exec_time_ns} ns")

    out = np.zeros((B, T, C), dtype=np.float32)
    for core in range(NCORES):
        out[core // 4] += res.results[core]["out"].astype(np.float32)
    return out


# revision 30
# speedup vs baseline: 1.0203x; 1.0048x over previous
"""Causal self-attention (B=2, T=2048, C=2048, H=16, D=128) on 8 trn2 cores.

Sharding: tensor-parallel over heads x data-parallel over batch.
Core c handles batch c//4, heads [4*(c%4) .. 4*(c%4)+4). Each core computes
qkv projection for its 4 heads, RoPE, causal attention, and a partial
output projection (its heads' rows of W_proj); the host sums the 4 partials
per batch.

All matmul operands are bf16 (fp32 PSUM accumulation): same PE streaming
rate as fp32r at N=512 but 4x faster weight loads (FWL) and half the DMA
traffic. q/k/v stay RESIDENT in SBUF between phases (48KB/partition) - no
DRAM spill round-trip. Weights are pre-transposed on the host so every DMA
is dense.

Phase 2 per head processes score-tile pairs [128,1024] (2 PSUM banks, one
ACT exp per pair), software-pipelined two pairs deep so the in-order PE
queue always has S matmuls to run while ACT exponentiates. Diagonal tiles
are width-restricted to the causally valid q-range (saves ~15% of phase-2
matmul columns and exp elements); the causal boundary inside a tile is a
single [128,128] additive triangle mask.
"""

import math
import os

import numpy as np

B, T, C = 2, 2048, 2048
H, D = 16, 128
HPC = 4  # heads per core
NCORES = 8

_CACHE = {}


def _build_program():
    import concourse.tile as tile
    from concourse import bacc, mybir

    f32 = mybir.dt.float32
    bf16 = mybir.dt.bfloat16
    Exp = mybir.ActivationFunctionType.Exp
    SCALE = 1.0 / math.sqrt(float(D))

    nc = bacc.Bacc(
        "TRN2", target_bir_lowering=False, debug=False, num_devices=NCORES
    )

    KT = C // 128  # 16 contraction tiles
    NTB = T // 512  # 4 t-blocks

    xT = nc.dram_tensor("xT", [C, T], bf16, kind="ExternalInput").ap()
    # [128, m, k*128]: per-m dense columns of the q|k weight blocks
    wqkT = nc.dram_tensor(
        "wqkT", [128, 8, KT * 128], bf16, kind="ExternalInput"
    ).ap()
    # [128, k*512]: per-k dense chunks of the v weight block
    wvT = nc.dram_tensor("wvT", [128, KT * 512], bf16, kind="ExternalInput").ap()
    wp = nc.dram_tensor("wp", [HPC * D, C], bf16, kind="ExternalInput").ap()
    onesr = nc.dram_tensor("onesr", [128, 128], bf16, kind="ExternalInput").ap()
    cosT = nc.dram_tensor("cosT", [128, T], bf16, kind="ExternalInput").ap()
    sinTs = nc.dram_tensor("sinTs", [128, T], bf16, kind="ExternalInput").ap()
    masktri = nc.dram_tensor("masktri", [128, 128], f32, kind="ExternalInput").ap()
    out = nc.dram_tensor("out", [T, C], bf16, kind="ExternalOutput").ap()

    with tile.TileContext(nc) as tc:
        with (
            tc.tile_pool(name="consts", bufs=1) as consts,
            tc.tile_pool(name="resid", bufs=1) as resid,
        ):
            # consts tiles are allocated here but their DMAs are emitted
            # inside phase 1, after the critical-path weight/x loads, so
            # they don't delay the first matmul chain
            cos_sb = consts.tile([128, T], bf16, tag="cos")
            sin_sb = consts.tile([128, T], bf16, tag="sin")
            ones_sb = consts.tile([128, 128], bf16, tag="ones")
            tri_sb = consts.tile([128, 128], f32, tag="tri")

            # q/k/v resident between phases: [128, m, T] for Q^T/K^T rows
            # (m 0-3 = q heads, 4-7 = k heads), [128, tchunk, 4*D] for V.
            qkres = resid.tile([128, 8, T], bf16, tag="qkres")
            vres = resid.tile([128, KT, HPC * D], bf16, tag="vres")
            o2 = [
                resid.tile([128, T], bf16, tag=f"o2_{h}", name=f"o2_{h}")
                for h in range(HPC)
            ]

            # ---------------- Phase 1: QKV projection ----------------
            with (
                tc.tile_pool(name="p1x", bufs=2) as p1x,
                tc.tile_pool(name="p1w", bufs=1) as p1w,
                tc.tile_pool(name="p1wv", bufs=1) as p1wv,
                tc.tile_pool(name="p1e", bufs=2) as p1e,
                tc.tile_pool(name="p1ps", bufs=2, space="PSUM") as p1ps,
            ):
                wqkg = p1w.tile([128, 8, KT * 128], bf16, tag="wqkg")
                wv_sb = p1wv.tile([128, KT * 512], bf16, tag="wv")
                xtb0 = p1x.tile([128, KT, 512], bf16, tag="xtb")
                MORD = (0, 4, 1, 5, 2, 6, 3, 7)

                def load_wm(m, eng=None):
                    (eng or nc.sync).dma_start(out=wqkg[:, m], in_=wqkT[:, m])

                # consts on the scalar queue (DVE needs cos/sin by the first
                # RoPE evacuation); weight/x critical path on the sync queue.
                nc.scalar.dma_start(out=cos_sb, in_=cosT)
                nc.scalar.dma_start(out=sin_sb, in_=sinTs)
                nc.scalar.dma_start(out=ones_sb, in_=onesr)
                nc.scalar.dma_start(out=tri_sb, in_=masktri)
                load_wm(MORD[0])
                load_wm(MORD[1])
                for k in range(KT):
                    nc.sync.dma_start(
                        out=xtb0[:, k], in_=xT[k * 128 : (k + 1) * 128, 0:512]
                    )
                    if k % 2 == 0 and k // 2 + 2 < 8:
                        load_wm(MORD[k // 2 + 2])
                for k in range(KT):
                    ks = slice(k * 512, (k + 1) * 512)
                    nc.sync.dma_start(out=wv_sb[:, ks], in_=wvT[:, ks])
                for tb in range(NTB):
                    tsl = slice(tb * 512, (tb + 1) * 512)
                    if tb == 0:
                        xtb = xtb0
                    else:
                        xtb = p1x.tile([128, KT, 512], bf16, tag="xtb",
                                       name="xtb")
                        for k in range(KT):
                            nc.sync.dma_start(
                                out=xtb[:, k],
                                in_=xT[k * 128 : (k + 1) * 128, tsl],
                            )
                    for m in MORD:
                        ps = p1ps.tile([128, 512], f32, tag="qk")
                        for k in range(KT):
                            nc.tensor.matmul(
                                ps,
                                lhsT=wqkg[:, m, k * 128 : (k + 1) * 128],
                                rhs=xtb[:, k, :],
                                start=(k == 0),
                                stop=(k == KT - 1),
                            )
                        # RoPE fused with PSUM evacuation, direct to qkres.
                        tmp = p1e.tile([128, 512], f32, tag="rtmp")
                        nc.vector.tensor_mul(
                            tmp[0:64], ps[64:128], sin_sb[0:64, tsl]
                        )
                        nc.vector.tensor_mul(
                            tmp[64:128], ps[0:64], sin_sb[64:128, tsl]
                        )
                        tmp2 = p1e.tile([128, 512], f32, tag="rtmp2")
                        nc.vector.tensor_mul(tmp2, ps, cos_sb[:, tsl])
                        nc.vector.tensor_add(qkres[:, m, tsl], tmp2, tmp)
                    for tsub in range(4):
                        csl = slice(tsub * 128, (tsub + 1) * 128)
                        psv = p1ps.tile([128, 512], f32, tag="v", bufs=2)
                        for k in range(KT):
                            nc.tensor.matmul(
                                psv,
                                lhsT=xtb[:, k, csl],
                                rhs=wv_sb[:, k * 512 : (k + 1) * 512],
                                start=(k == 0),
                                stop=(k == KT - 1),
                            )
                        nc.scalar.copy(vres[:, tb * 4 + tsub, :], psv)

            # ---------------- Phase 2: attention ----------------
            with tc.tile_pool(name="p3w", bufs=1) as p3w:
                wps = [
                    p3w.tile([128, T], bf16, tag=f"wp{i}", name=f"wp{i}")
                    for i in range(HPC)
                ]
                for i in range(HPC):
                    nc.sync.dma_start(
                        out=wps[i], in_=wp[i * 128 : (i + 1) * 128, :]
                    )
                _phase2(tc, nc, f32, bf16, Exp, SCALE, qkres, vres,
                        tri_sb, ones_sb, o2)
                _phase3(tc, nc, f32, bf16, o2, wps, out)
    nc.compile()
    return nc


def _phase2(tc, nc, f32, bf16, Exp, SCALE, qkres, vres, tri_sb, ones_sb, o2):
    """Score pairs software-pipelined 2 deep: S matmuls (and exp) for pair
    j+2 are emitted before pv/dn matmuls of pair j, so the in-order PE
    stream never waits on ACT."""
    with (
        tc.tile_pool(name="p2st", bufs=3, space="PSUM") as stp,
        tc.tile_pool(name="p2pt", bufs=3) as ptp,
        tc.tile_pool(name="p2s", bufs=2) as p2s,
        tc.tile_pool(name="p2pv", bufs=1, space="PSUM") as pvp,
        tc.tile_pool(name="p2dn", bufs=1, space="PSUM") as dnp,
    ):
        # flat pair list across heads and q-blocks
        pairs = []
        for h in range(HPC):
            for qb in range(4):
                nk = 4 * (qb + 1)
                for jp in range(nk // 2):
                    pairs.append((h, qb, nk, 2 * jp, 2 * jp + 1))

        def width(qb, kb):
            j = kb - 4 * qb
            return 512 if j < 0 else 512 - 128 * j

        queue = []
        state = {}  # (h, qb) -> (pv, dn)
        for idx in range(len(pairs) + 2):
            if idx < len(pairs):
                h, qb, nk, kb0, kb1 = pairs[idx]
                w0, w1 = width(qb, kb0), width(qb, kb1)
                st = stp.tile([128, 1024], f32, tag="st")
                pt = ptp.tile([128, 1024], bf16, tag="pt")
                for kb, w, o in ((kb0, w0, 0), (kb1, w1, w0)):
                    off = 512 - w
                    nc.tensor.matmul(
                        st[:, o : o + w],
                        lhsT=qkres[:, 4 + h, kb * 128 : (kb + 1) * 128],
                        rhs=qkres[:, h, qb * 512 + off : (qb + 1) * 512],
                        start=True,
                        stop=True,
                    )
                    if kb >= 4 * qb:  # diagonal: causal triangle mask
                        nc.vector.tensor_add(
                            st[:, o : o + 128], st[:, o : o + 128], tri_sb
                        )
                nc.scalar.activation(
                    pt[:, 0 : w0 + w1], st[:, 0 : w0 + w1], Exp, scale=SCALE
                )
                queue.append((h, qb, nk, pt, ((kb0, w0, 0), (kb1, w1, w0))))
            if idx >= 2:
                h, qb, nk, pt, halves = queue[idx - 2]
                if halves[0][0] == 0:  # first pair of this q-block
                    state[(h, qb)] = (
                        pvp.tile([128, 512], f32, tag="pv",
                                 name=f"pv{h}_{qb}"),
                        dnp.tile([128, 512], f32, tag="dn",
                                 name=f"dn{h}_{qb}"),
                    )
                pv, dn = state[(h, qb)]
                for kb, w, o in halves:
                    off = 512 - w
                    nc.tensor.matmul(
                        dn[:, off:512],
                        lhsT=ones_sb,
                        rhs=pt[:, o : o + w],
                        start=(kb == 0),
                        stop=(kb == nk - 1),
                    )
                    nc.tensor.matmul(
                        pv[:, off:512],
                        lhsT=vres[:, kb, h * 128 : (h + 1) * 128],
                        rhs=pt[:, o : o + w],
                        start=(kb == 0),
                        stop=(kb == nk - 1),
                    )
                if halves[1][0] == nk - 1:  # last pair: normalize + evac
                    qsl = slice(qb * 512, (qb + 1) * 512)
                    rb2 = p2s.tile([128, 512], f32, tag="rb2")
                    nc.vector.reciprocal_approx_fast(out=rb2, in_=dn)
                    nc.vector.tensor_mul(o2[h][:, qsl], pv, rb2)


def _phase3(tc, nc, f32, bf16, o2, wps, out):
    with (
        tc.tile_pool(name="p3e", bufs=4) as p3e,
        tc.tile_pool(name="p3ps", bufs=8, space="PSUM") as p3ps,
    ):
        NT = T // 128
        for t in range(NT):
            tsl = slice(t * 128, (t + 1) * 128)
            pos = [
                p3ps.tile([128, 512], f32, tag="po", name=f"po{t}_{cb}")
                for cb in range(4)
            ]
            if t < NT - 1:
                # hd outer / cb inner: 4 matmuls share one LDWEIGHTS.
                for hd in range(HPC):
                    for cb in range(4):
                        nc.tensor.matmul(
                            pos[cb],
                            lhsT=o2[hd][:, tsl],
                            rhs=wps[hd][:, cb * 512 : (cb + 1) * 512],
                            start=(hd == 0),
                            stop=(hd == HPC - 1),
                        )
                for cb in range(4):
                    ob = p3e.tile([128, 512], bf16, tag="ob")
                    # alternate evacuation engine: DVE / ACT
                    if cb % 2 == 0:
                        nc.vector.tensor_copy(ob, pos[cb])
                    else:
                        nc.scalar.copy(ob, pos[cb])
                    (nc.sync if cb % 2 == 0 else nc.scalar).dma_start(
                        out=out[tsl, cb * 512 : (cb + 1) * 512], in_=ob
                    )
            else:
                # last tile: cb outer so each pos finishes early and its
                # evacuation + DMA overlap the remaining matmuls (short tail)
                for cb in range(4):
                    for hd in range(HPC):
                        nc.tensor.matmul(
                            pos[cb],
                            lhsT=o2[hd][:, tsl],
                            rhs=wps[hd][:, cb * 512 : (cb + 1) * 512],
                            start=(hd == 0),
                            stop=(hd == HPC - 1),
                        )
                    ob = p3e.tile([128, 512], bf16, tag="ob")
                    if cb % 2 == 0:
                        nc.vector.tensor_copy(ob, pos[cb])
                    else:
                        nc.scalar.copy(ob, pos[cb])
                    (nc.sync if cb % 2 == 0 else nc.scalar).dma_start(
                        out=out[tsl, cb * 512 : (cb + 1) * 512], in_=ob
                    )


def _get_program():
    if "nc" not in _CACHE:
        _CACHE["nc"] = _build_program()
    return _CACHE["nc"]


def make_in_maps(x, cos, sin, W_qkv, W_proj):
    """Host-side sharding: per-core input dicts (numpy)."""
    import ml_dtypes

    bf16 = ml_dtypes.bfloat16
    KT = C // 128
    x = np.asarray(x, dtype=np.float32)
    cos = np.asarray(cos, dtype=np.float32)
    sin = np.asarray(sin, dtype=np.float32)
    W_qkv = np.asarray(W_qkv, dtype=np.float32)
    W_proj = np.asarray(W_proj, dtype=np.float32)

    cosT = np.ascontiguousarray(np.tile(cos.T, (2, 1)).astype(bf16))  # [128, T]
    sinT = np.ascontiguousarray(
        np.concatenate([-sin.T, sin.T], axis=0).astype(bf16)
    )
    k_idx = np.arange(128)[:, None]
    q_idx = np.arange(128)[None, :]
    masktri = np.where(q_idx >= k_idx, 0.0, -1.0e30).astype(np.float32)
    onesr = np.ones((128, 128), dtype=bf16)

    in_maps = []
    for core in range(NCORES):
        b, hg = core // 4, core % 4
        csl = slice(hg * 512, (hg + 1) * 512)
        # q|k columns for this head group: [C, 1024]
        wqk_np = np.concatenate(
            [W_qkv[:, csl], W_qkv[:, C + hg * 512 : C + (hg + 1) * 512]],
            axis=1,
        )
        # -> [128(p), 8(m), KT*128] with rows split as (k,p)
        wqkT_np = np.ascontiguousarray(
            wqk_np.reshape(KT, 128, 8, 128)
            .transpose(1, 2, 0, 3)
            .reshape(128, 8, KT * 128)
            .astype(bf16)
        )
        wv_np = W_qkv[:, 2 * C + hg * 512 : 2 * C + (hg + 1) * 512]
        wvT_np = np.ascontiguousarray(
            wv_np.reshape(KT, 128, 512)
            .transpose(1, 0, 2)
            .reshape(128, KT * 512)
            .astype(bf16)
        )
        wp_np = np.ascontiguousarray(
            W_proj[hg * 512 : (hg + 1) * 512, :].astype(bf16)
        )
        xT_np = np.ascontiguousarray(x[b].T.astype(bf16))
        in_maps.append(
            {
                "xT": xT_np,
                "wqkT": wqkT_np,
                "wvT": wvT_np,
                "wp": wp_np,
                "onesr": onesr,
                "cosT": cosT,
                "sinTs": sinT,
                "masktri": masktri,
            }
        )
    return in_maps


def kernel(x, cos, sin, W_qkv, W_proj):
    from concourse.bass_utils import run_bass_kernel_spmd

    nc = _get_program()
    in_maps = make_in_maps(x, cos, sin, W_qkv, W_proj)
    trace = bool(int(os.environ.get("KERNEL_TRACE", "0")))
    res = run_bass_kernel_spmd(
        nc, in_maps, core_ids=list(range(NCORES)), trace=trace
    )
    if trace:
        _CACHE["last_results"] = res
        if res.exec_time_ns is not None:
            print(f"HW exec time: {res.exec_time_ns} ns")

    out = np.zeros((B, T, C), dtype=np.float32)
    for core in range(NCORES):
        out[core // 4] += res.results[core]["out"].astype(np.float32)
    return out


# revision 31
# speedup vs baseline: 1.0251x; 1.0047x over previous
"""Causal self-attention (B=2, T=2048, C=2048, H=16, D=128) on 8 trn2 cores.

Sharding: tensor-parallel over heads x data-parallel over batch.
Core c handles batch c//4, heads [4*(c%4) .. 4*(c%4)+4). Each core computes
qkv projection for its 4 heads, RoPE, causal attention, and a partial
output projection (its heads' rows of W_proj); the host sums the 4 partials
per batch.

All matmul operands are bf16 (fp32 PSUM accumulation): same PE streaming
rate as fp32r at N=512 but 4x faster weight loads (FWL) and half the DMA
traffic. q/k/v stay RESIDENT in SBUF between phases (48KB/partition) - no
DRAM spill round-trip. Weights are pre-transposed on the host so every DMA
is dense.

Phase 2 per head processes score-tile pairs [128,1024] (2 PSUM banks, one
ACT exp per pair), software-pipelined two pairs deep so the in-order PE
queue always has S matmuls to run while ACT exponentiates. Diagonal tiles
are width-restricted to the causally valid q-range (saves ~15% of phase-2
matmul columns and exp elements); the causal boundary inside a tile is a
single [128,128] additive triangle mask.
"""

import math
import os

import numpy as np

B, T, C = 2, 2048, 2048
H, D = 16, 128
HPC = 4  # heads per core
NCORES = 8

_CACHE = {}


def _build_program():
    import concourse.tile as tile
    from concourse import bacc, mybir

    f32 = mybir.dt.float32
    bf16 = mybir.dt.bfloat16
    Exp = mybir.ActivationFunctionType.Exp
    SCALE = 1.0 / math.sqrt(float(D))

    nc = bacc.Bacc(
        "TRN2", target_bir_lowering=False, debug=False, num_devices=NCORES
    )

    KT = C // 128  # 16 contraction tiles
    NTB = T // 512  # 4 t-blocks

    xT = nc.dram_tensor("xT", [C, T], bf16, kind="ExternalInput").ap()
    # [128, m, k*128]: per-m dense columns of the q|k weight blocks
    wqkT = nc.dram_tensor(
        "wqkT", [128, 8, KT * 128], bf16, kind="ExternalInput"
    ).ap()
    # [128, k*512]: per-k dense chunks of the v weight block
    wvT = nc.dram_tensor("wvT", [128, KT * 512], bf16, kind="ExternalInput").ap()
    wp = nc.dram_tensor("wp", [HPC * D, C], bf16, kind="ExternalInput").ap()
    onesr = nc.dram_tensor("onesr", [128, 128], bf16, kind="ExternalInput").ap()
    cosT = nc.dram_tensor("cosT", [128, T], bf16, kind="ExternalInput").ap()
    sinTs = nc.dram_tensor("sinTs", [128, T], bf16, kind="ExternalInput").ap()
    masktri = nc.dram_tensor("masktri", [128, 128], f32, kind="ExternalInput").ap()
    out = nc.dram_tensor("out", [T, C], bf16, kind="ExternalOutput").ap()

    with tile.TileContext(nc) as tc:
        with (
            tc.tile_pool(name="consts", bufs=1) as consts,
            tc.tile_pool(name="resid", bufs=1) as resid,
        ):
            # consts tiles are allocated here but their DMAs are emitted
            # inside phase 1, after the critical-path weight/x loads, so
            # they don't delay the first matmul chain
            cos_sb = consts.tile([128, T], bf16, tag="cos")
            sin_sb = consts.tile([128, T], bf16, tag="sin")
            ones_sb = consts.tile([128, 128], bf16, tag="ones")
            tri_sb = consts.tile([128, 128], f32, tag="tri")

            # q/k/v resident between phases: [128, m, T] for Q^T/K^T rows
            # (m 0-3 = q heads, 4-7 = k heads), [128, tchunk, 4*D] for V.
            qkres = resid.tile([128, 8, T], bf16, tag="qkres")
            vres = resid.tile([128, KT, HPC * D], bf16, tag="vres")
            o2 = [
                resid.tile([128, T], bf16, tag=f"o2_{h}", name=f"o2_{h}")
                for h in range(HPC)
            ]

            # ---------------- Phase 1: QKV projection ----------------
            with (
                tc.tile_pool(name="p1x", bufs=2) as p1x,
                tc.tile_pool(name="p1w", bufs=1) as p1w,
                tc.tile_pool(name="p1wv", bufs=1) as p1wv,
                tc.tile_pool(name="p1e", bufs=2) as p1e,
                tc.tile_pool(name="p1ps", bufs=2, space="PSUM") as p1ps,
            ):
                wqkg = p1w.tile([128, 8, KT * 128], bf16, tag="wqkg")
                wv_sb = p1wv.tile([128, KT * 512], bf16, tag="wv")
                xtb0 = p1x.tile([128, KT, 512], bf16, tag="xtb")
                MORD = (0, 4, 1, 5, 2, 6, 3, 7)

                def load_wm(m, eng=None):
                    (eng or nc.sync).dma_start(out=wqkg[:, m], in_=wqkT[:, m])

                # consts on the scalar queue (DVE needs cos/sin by the first
                # RoPE evacuation); weight/x critical path on the sync queue.
                nc.scalar.dma_start(out=cos_sb, in_=cosT)
                nc.scalar.dma_start(out=sin_sb, in_=sinTs)
                nc.scalar.dma_start(out=ones_sb, in_=onesr)
                nc.scalar.dma_start(out=tri_sb, in_=masktri)
                load_wm(MORD[0])
                load_wm(MORD[1])
                for k in range(KT):
                    nc.sync.dma_start(
                        out=xtb0[:, k], in_=xT[k * 128 : (k + 1) * 128, 0:512]
                    )
                    if k % 2 == 0 and k // 2 + 2 < 8:
                        load_wm(MORD[k // 2 + 2])
                for k in range(KT):
                    ks = slice(k * 512, (k + 1) * 512)
                    nc.sync.dma_start(out=wv_sb[:, ks], in_=wvT[:, ks])
                for tb in range(NTB):
                    tsl = slice(tb * 512, (tb + 1) * 512)
                    if tb == 0:
                        xtb = xtb0
                    else:
                        xtb = p1x.tile([128, KT, 512], bf16, tag="xtb",
                                       name="xtb")
                        for k in range(KT):
                            nc.sync.dma_start(
                                out=xtb[:, k],
                                in_=xT[k * 128 : (k + 1) * 128, tsl],
                            )
                    for m in MORD:
                        ps = p1ps.tile([128, 512], f32, tag="qk")
                        for k in range(KT):
                            nc.tensor.matmul(
                                ps,
                                lhsT=wqkg[:, m, k * 128 : (k + 1) * 128],
                                rhs=xtb[:, k, :],
                                start=(k == 0),
                                stop=(k == KT - 1),
                            )
                        # RoPE fused with PSUM evacuation, direct to qkres.
                        tmp = p1e.tile([128, 512], f32, tag="rtmp")
                        nc.vector.tensor_mul(
                            tmp[0:64], ps[64:128], sin_sb[0:64, tsl]
                        )
                        nc.vector.tensor_mul(
                            tmp[64:128], ps[0:64], sin_sb[64:128, tsl]
                        )
                        tmp2 = p1e.tile([128, 512], f32, tag="rtmp2")
                        nc.vector.tensor_mul(tmp2, ps, cos_sb[:, tsl])
                        nc.vector.tensor_add(qkres[:, m, tsl], tmp2, tmp)
                    for tsub in range(4):
                        csl = slice(tsub * 128, (tsub + 1) * 128)
                        psv = p1ps.tile([128, 512], f32, tag="v", bufs=2)
                        for k in range(KT):
                            nc.tensor.matmul(
                                psv,
                                lhsT=xtb[:, k, csl],
                                rhs=wv_sb[:, k * 512 : (k + 1) * 512],
                                start=(k == 0),
                                stop=(k == KT - 1),
                            )
                        nc.scalar.copy(vres[:, tb * 4 + tsub, :], psv)

            # ---------------- Phase 2: attention ----------------
            with tc.tile_pool(name="p3w", bufs=1) as p3w:
                wps = [
                    p3w.tile([128, T], bf16, tag=f"wp{i}", name=f"wp{i}")
                    for i in range(HPC)
                ]
                for i in range(HPC):
                    nc.scalar.dma_start(
                        out=wps[i], in_=wp[i * 128 : (i + 1) * 128, :]
                    )
                _phase2(tc, nc, f32, bf16, Exp, SCALE, qkres, vres,
                        tri_sb, ones_sb, o2)
                _phase3(tc, nc, f32, bf16, o2, wps, out)
    nc.compile()
    return nc


def _phase2(tc, nc, f32, bf16, Exp, SCALE, qkres, vres, tri_sb, ones_sb, o2):
    """Score pairs software-pipelined 2 deep: S matmuls (and exp) for pair
    j+2 are emitted before pv/dn matmuls of pair j, so the in-order PE
    stream never waits on ACT."""
    with (
        tc.tile_pool(name="p2st", bufs=3, space="PSUM") as stp,
        tc.tile_pool(name="p2pt", bufs=4) as ptp,
        tc.tile_pool(name="p2s", bufs=2) as p2s,
        tc.tile_pool(name="p2pv", bufs=1, space="PSUM") as pvp,
        tc.tile_pool(name="p2dn", bufs=1, space="PSUM") as dnp,
    ):
        # flat pair list across heads and q-blocks
        pairs = []
        for h in range(HPC):
            for qb in range(4):
                nk = 4 * (qb + 1)
                for jp in range(nk // 2):
                    pairs.append((h, qb, nk, 2 * jp, 2 * jp + 1))

        def width(qb, kb):
            j = kb - 4 * qb
            return 512 if j < 0 else 512 - 128 * j

        queue = []
        state = {}  # (h, qb) -> (pv, dn)
        DEPTH = 3  # pv/dn trail the S/exp emission by this many pairs
        for idx in range(len(pairs) + DEPTH):
            if idx < len(pairs):
                h, qb, nk, kb0, kb1 = pairs[idx]
                w0, w1 = width(qb, kb0), width(qb, kb1)
                st = stp.tile([128, 1024], f32, tag="st")
                pt = ptp.tile([128, 1024], bf16, tag="pt")
                for kb, w, o in ((kb0, w0, 0), (kb1, w1, w0)):
                    off = 512 - w
                    nc.tensor.matmul(
                        st[:, o : o + w],
                        lhsT=qkres[:, 4 + h, kb * 128 : (kb + 1) * 128],
                        rhs=qkres[:, h, qb * 512 + off : (qb + 1) * 512],
                        start=True,
                        stop=True,
                    )
                    if kb >= 4 * qb:  # diagonal: causal triangle mask
                        nc.vector.tensor_add(
                            st[:, o : o + 128], st[:, o : o + 128], tri_sb
                        )
                nc.scalar.activation(
                    pt[:, 0 : w0 + w1], st[:, 0 : w0 + w1], Exp, scale=SCALE
                )
                queue.append((h, qb, nk, pt, ((kb0, w0, 0), (kb1, w1, w0))))
            if idx >= DEPTH:
                h, qb, nk, pt, halves = queue[idx - DEPTH]
                if halves[0][0] == 0:  # first pair of this q-block
                    state[(h, qb)] = (
                        pvp.tile([128, 512], f32, tag="pv",
                                 name=f"pv{h}_{qb}"),
                        dnp.tile([128, 512], f32, tag="dn",
                                 name=f"dn{h}_{qb}"),
                    )
                pv, dn = state[(h, qb)]
                for kb, w, o in halves:
                    off = 512 - w
                    nc.tensor.matmul(
                        dn[:, off:512],
                        lhsT=ones_sb,
                        rhs=pt[:, o : o + w],
                        start=(kb == 0),
                        stop=(kb == nk - 1),
                    )
                    nc.tensor.matmul(
                        pv[:, off:512],
                        lhsT=vres[:, kb, h * 128 : (h + 1) * 128],
                        rhs=pt[:, o : o + w],
                        start=(kb == 0),
                        stop=(kb == nk - 1),
                    )
                if halves[1][0] == nk - 1:  # last pair: normalize + evac
                    qsl = slice(qb * 512, (qb + 1) * 512)
                    rb2 = p2s.tile([128, 512], f32, tag="rb2")
                    nc.vector.reciprocal_approx_fast(out=rb2, in_=dn)
                    nc.vector.tensor_mul(o2[h][:, qsl], pv, rb2)


def _phase3(tc, nc, f32, bf16, o2, wps, out):
    with (
        tc.tile_pool(name="p3e", bufs=4) as p3e,
        tc.tile_pool(name="p3ps", bufs=8, space="PSUM") as p3ps,
    ):
        NT = T // 128
        for t in range(NT):
            tsl = slice(t * 128, (t + 1) * 128)
            pos = [
                p3ps.tile([128, 512], f32, tag="po", name=f"po{t}_{cb}")
                for cb in range(4)
            ]
            if t < NT - 1:
                # hd outer / cb inner: 4 matmuls share one LDWEIGHTS.
                for hd in range(HPC):
                    for cb in range(4):
                        nc.tensor.matmul(
                            pos[cb],
                            lhsT=o2[hd][:, tsl],
                            rhs=wps[hd][:, cb * 512 : (cb + 1) * 512],
                            start=(hd == 0),
                            stop=(hd == HPC - 1),
                        )
                for cb in range(4):
                    ob = p3e.tile([128, 512], bf16, tag="ob")
                    # alternate evacuation engine: DVE / ACT
                    if cb % 2 == 0:
                        nc.vector.tensor_copy(ob, pos[cb])
                    else:
                        nc.scalar.copy(ob, pos[cb])
                    (nc.sync if cb % 2 == 0 else nc.scalar).dma_start(
                        out=out[tsl, cb * 512 : (cb + 1) * 512], in_=ob
                    )
            else:
                # last tile: cb outer so each pos finishes early and its
                # evacuation + DMA overlap the remaining matmuls (short tail)
                for cb in range(4):
                    for hd in range(HPC):
                        nc.tensor.matmul(
                            pos[cb],
                            lhsT=o2[hd][:, tsl],
                            rhs=wps[hd][:, cb * 512 : (cb + 1) * 512],
                            start=(hd == 0),
                            stop=(hd == HPC - 1),
                        )
                    ob = p3e.tile([128, 512], bf16, tag="ob")
                    if cb % 2 == 0:
                        nc.vector.tensor_copy(ob, pos[cb])
                    else:
                        nc.scalar.copy(ob, pos[cb])
                    (nc.sync if cb % 2 == 0 else nc.scalar).dma_start(
                        out=out[tsl, cb * 512 : (cb + 1) * 512], in_=ob
                    )


def _get_program():
    if "nc" not in _CACHE:
        _CACHE["nc"] = _build_program()
    return _CACHE["nc"]


def make_in_maps(x, cos, sin, W_qkv, W_proj):
    """Host-side sharding: per-core input dicts (numpy)."""
    import ml_dtypes

    bf16 = ml_dtypes.bfloat16
    KT = C // 128
    x = np.asarray(x, dtype=np.float32)
    cos = np.asarray(cos, dtype=np.float32)
    sin = np.asarray(sin, dtype=np.float32)
    W_qkv = np.asarray(W_qkv, dtype=np.float32)
    W_proj = np.asarray(W_proj, dtype=np.float32)

    cosT = np.ascontiguousarray(np.tile(cos.T, (2, 1)).astype(bf16))  # [128, T]
    sinT = np.ascontiguousarray(
        np.concatenate([-sin.T, sin.T], axis=0).astype(bf16)
    )
    k_idx = np.arange(128)[:, None]
    q_idx = np.arange(128)[None, :]
    masktri = np.where(q_idx >= k_idx, 0.0, -1.0e30).astype(np.float32)
    onesr = np.ones((128, 128), dtype=bf16)

    in_maps = []
    for core in range(NCORES):
        b, hg = core // 4, core % 4
        csl = slice(hg * 512, (hg + 1) * 512)
        # q|k columns for this head group: [C, 1024]
        wqk_np = np.concatenate(
            [W_qkv[:, csl], W_qkv[:, C + hg * 512 : C + (hg + 1) * 512]],
            axis=1,
        )
        # -> [128(p), 8(m), KT*128] with rows split as (k,p)
        wqkT_np = np.ascontiguousarray(
            wqk_np.reshape(KT, 128, 8, 128)
            .transpose(1, 2, 0, 3)
            .reshape(128, 8, KT * 128)
            .astype(bf16)
        )
        wv_np = W_qkv[:, 2 * C + hg * 512 : 2 * C + (hg + 1) * 512]
        wvT_np = np.ascontiguousarray(
            wv_np.reshape(KT, 128, 512)
            .transpose(1, 0, 2)
            .reshape(128, KT * 512)
            .astype(bf16)
        )
        wp_np = np.ascontiguousarray(
            W_proj[hg * 512 : (hg + 1) * 512, :].astype(bf16)
        )
        xT_np = np.ascontiguousarray(x[b].T.astype(bf16))
        in_maps.append(
            {
                "xT": xT_np,
                "wqkT": wqkT_np,
                "wvT": wvT_np,
                "wp": wp_np,
                "onesr": onesr,
                "cosT": cosT,
                "sinTs": sinT,
                "masktri": masktri,
            }
        )
    return in_maps


def kernel(x, cos, sin, W_qkv, W_proj):
    from concourse.bass_utils import run_bass_kernel_spmd

    nc = _get_program()
    in_maps = make_in_maps(x, cos, sin, W_qkv, W_proj)
    trace = bool(int(os.environ.get("KERNEL_TRACE", "0")))
    res = run_bass_kernel_spmd(
        nc, in_maps, core_ids=list(range(NCORES)), trace=trace
    )
    if trace:
        _CACHE["last_results"] = res
        if res.exec_time_ns is not None:
            print(f"HW exec time: {res.exec_time_ns} ns")

    out = np.zeros((B, T, C), dtype=np.float32)
    for core in range(NCORES):
        out[core // 4] += res.results[core]["out"].astype(np.float32)
    return out
